# revision 2
# baseline (speedup 1.0000x reference)
"""Trainium2 Bass kernel for nn_AttentionLayer (B=4, C=64, N=4096, dk=64).

Math (per batch b):
    q_t[d, n] = (Wq/8) @ x[b]          # [64, N]
    k[d, m]   = Wk @ x[b]              # [64, N]
    v_t[n, o] = (Wv @ x[b]).T          # [N, 64]
    s[n, m]   = q_t.T @ k              # [N, N]
    attn      = softmax over n (columns)
    out[o, m] = v.T @ attn             # [64, N]

Sharding: 8 cores = 4 batches x 2 column-halves; core (b, h) computes
out[b, :, h*2048:(h+1)*2048]. The softmax axis n is fully local -> no
collectives. The tiny projections (0.25% of FLOPs) run on host so the
device inputs can be fed pre-laid-out in the matmul dtypes.

Device kernel per core (the N^2 part), fp16 compute (1 cyc/row on the PE;
float32r measures ~4 cyc/row on this hardware despite the cost model):
  - scores: TensorE fp16 matmuls [128x512] into [128, 1536] PSUM groups
    (3 banks, double-buffered so TensorE isn't WAR-blocked on ScalarE),
    lhsT = q_t chunk, rhs = k slice
  - exp: ScalarE straight out of PSUM -> fp16 SBUF (no max-subtraction:
    scores are O(+-6), fp32 PSUM and fp16 exp are safe)
  - AV: lhsT = v_t chunk [128, 65] fp16 (65th col = ones -> colsum),
    rhs = exp chunk, accumulated into PSUM [65, 512] f32 over 32 chunks
  - out DMA [65, 2048]: rows 0:64 = numerator, row 64 = colsum.
Host divides numerator by colsum and reassembles the full output.

Measured (loop-slope method, see bench.py): ~89.7 us/core on hardware,
rel_err 2.1e-04 vs the f64 reference.
"""

import ml_dtypes
import numpy as np

import concourse.bass as bass  # noqa: F401  (registers engine methods)
import concourse.mybir as mybir
import concourse.tile as tile
from concourse import bacc
from concourse.bass_utils import run_bass_kernel_spmd

B, C, N = 4, 64, 4096
MLOC = N // 2            # columns per core
P = 128
NCH = N // P             # 32 row-chunks of the score matrix
MT = 512                 # m-tile width (PSUM free dim)
NMT = MLOC // MT         # 4 m-tiles per core
GRP = 4                  # score chunks exp'd per ScalarE instruction
CP1 = C + 1              # v columns + ones column

F32 = mybir.dt.float32
F32R = mybir.dt.float32r
BF16 = mybir.dt.bfloat16
FP16 = mybir.dt.float16
EXP = mybir.ActivationFunctionType.Exp

_NC_CACHE = {}


def _build(grp=GRP, spsum_bufs=1, exp_bufs=2, prec="f32r", staged=False,
           dma_split=False, loop_reps=None):
    """Build the per-core graph.

    grp: score chunks per exp instruction ([128, grp*512] PSUM group).
    spsum_bufs: score-PSUM group buffers (grp*spsum_bufs + 2 <= 8 banks).
    exp_bufs: exp_sb SBUF buffers (32KB/partition each).
    prec: low-precision dtype for q/k (scores matmul) and v/exp (AV matmul).
        "f32r": q/k float32r, v/exp bf16. NOTE: float32r measures ~4 cyc/row
        on this hardware (the cost model wrongly says 1) -- do not ship.
        "bf16": all bf16 (1 cyc/row). "fp16": all float16 (1 cyc/row, 10
        mantissa bits -> ~8x less rounding error than bf16).
    staged: DVE-copy scores PSUM->SBUF half-tiles [128, 8192] and run exp
        from SBUF in 8 giant ScalarE instructions (grp ignored; PSUM =
        [128,1024]x3 + 2 AV banks). Targets slow ACT-from-PSUM reads.
    loop_reps: if set, wrap the attention body in a hardware For_i loop
        (used only for timing: per-iteration time = slope over reps).
    """
    if staged:
        grp, spsum_bufs = 2, 3
    assert grp * spsum_bufs + 2 <= 8
    qk_dt = {"f32r": F32R, "bf16": BF16, "fp16": FP16}[prec]
    lp_dt = {"f32r": BF16, "bf16": BF16, "fp16": FP16}[prec]
    nc = bacc.Bacc("TRN2", target_bir_lowering=False, debug=False)
    q_ext = nc.declare_dram_parameter("q", [C, N], qk_dt, isOutput=False)
    k_ext = nc.declare_dram_parameter("k", [C, MLOC], qk_dt, isOutput=False)
    v_ext = nc.declare_dram_parameter("v", [P, NCH * CP1], lp_dt, isOutput=False)
    out_ext = nc.declare_dram_parameter("out", [CP1, MLOC], F32, isOutput=True)

    # n-chunk groups per m-tile, e.g. grp=3 -> [3]*10 + [2]
    gsizes = []
    left = NCH
    while left > 0:
        gsizes.append(min(grp, left))
        left -= gsizes[-1]

    with tile.TileContext(nc) as tc:
        with (
            tc.tile_pool(name="const", bufs=1) as cpool,
            tc.tile_pool(name="expp", bufs=exp_bufs) as epool,
            tc.tile_pool(name="outp", bufs=2) as opool,
            tc.tile_pool(name="stg", bufs=2) as gpool,
            tc.tile_pool(name="spsum", bufs=spsum_bufs, space="PSUM") as spool,
            tc.tile_pool(name="apsum", bufs=2, space="PSUM") as apool,
        ):
            # One serial HWDGE queue -> emit in first-needed order: the first
            # scores group needs q[:, :384] and k[:, :512]; v is needed ~3us
            # in (first AV matmul); later k/q chunks are consumed much later.
            k_sb = cpool.tile([C, MLOC], qk_dt)
            q_sb = cpool.tile([C, N], qk_dt)
            v_sb = cpool.tile([P, NCH * CP1], lp_dt)
            vw = NCH * CP1 // 4

            def dq(j, eng=nc.sync):
                eng.dma_start(
                    q_sb[:, j * 512:(j + 1) * 512], q_ext[:, j * 512:(j + 1) * 512]
                )

            def dk(j, eng=nc.sync):
                eng.dma_start(
                    k_sb[:, j * 512:(j + 1) * 512], k_ext[:, j * 512:(j + 1) * 512]
                )

            def dv(j, eng=nc.sync):
                eng.dma_start(
                    v_sb[:, j * vw:(j + 1) * vw], v_ext[:, j * vw:(j + 1) * vw]
                )

            if dma_split:
                # two queues: sync(HWDGE) feeds the critical path (q, k0);
                # gpsimd(SWDGE) streams v and the k tail in parallel
                dq(0); dk(0); dq(1); dq(2); dq(3); dq(4); dq(5); dq(6); dq(7)
                for j in range(4):
                    dv(j, nc.gpsimd)
                for j in (1, 2, 3):
                    dk(j, nc.gpsimd)
            else:
                dq(0); dk(0); dq(1); dv(0); dq(2); dv(1); dq(3); dv(2)
                dq(4); dv(3); dq(5); dq(6); dq(7); dk(1); dk(2); dk(3)

            def q_ap(i):
                return q_sb[:, i * P:(i + 1) * P]

            def k_ap(t):
                return k_sb[:, t * MT:(t + 1) * MT]

            def attention_body(iv=None):
                for t in range(NMT):
                    exp_sb = epool.tile([P, NCH * MT], lp_dt, tag="exp")
                    if staged:
                        # 2 halves of 16 chunks: PE -> psum [128,1024] (2
                        # chunks) -> DVE copy -> s_half SBUF -> one giant exp
                        for h in range(2):
                            s_half = gpool.tile([P, 16 * MT], lp_dt, tag="sh")
                            for j in range(8):
                                ps = spool.tile([P, 2 * MT], F32, tag="sc")
                                for u in range(2):
                                    i = h * 16 + 2 * j + u
                                    nc.tensor.matmul(
                                        ps[:, u * MT:(u + 1) * MT],
                                        lhsT=q_ap(i),
                                        rhs=k_ap(t),
                                        start=True,
                                        stop=True,
                                    )
                                nc.vector.tensor_copy(
                                    s_half[:, j * 2 * MT:(j + 1) * 2 * MT], ps[:]
                                )
                            nc.scalar.activation(
                                exp_sb[:, h * 16 * MT:(h + 1) * 16 * MT],
                                s_half[:],
                                EXP,
                            )
                    else:
                        i = 0
                        for gs in gsizes:
                            ps = spool.tile([P, grp * MT], F32, tag="sc")
                            for u in range(gs):
                                nc.tensor.matmul(
                                    ps[:, u * MT:(u + 1) * MT],
                                    lhsT=q_ap(i + u),
                                    rhs=k_ap(t),
                                    start=True,
                                    stop=True,
                                )
                            nc.scalar.activation(
                                exp_sb[:, i * MT:(i + gs) * MT], ps[:, :gs * MT], EXP
                            )
                            i += gs
                    pav = apool.tile([CP1, MT], F32, tag="av")
                    for i in range(NCH):
                        nc.tensor.matmul(
                            pav[:],
                            lhsT=v_sb[:, i * CP1:(i + 1) * CP1],
                            rhs=exp_sb[:, i * MT:(i + 1) * MT],
                            start=(i == 0),
                            stop=(i == NCH - 1),
                        )
                    o_sb = opool.tile([CP1, MT], F32, tag="ot")
                    nc.vector.tensor_copy(o_sb[:], pav[:])
                    nc.sync.dma_start(out_ext[:, t * MT:(t + 1) * MT], o_sb[:])

            if loop_reps is None:
                attention_body()
            else:
                with tc.For_i(0, loop_reps, 1):
                    attention_body()

    nc.compile()
    return nc


BEST = {"grp": 3, "spsum_bufs": 2, "prec": "fp16"}


def _in_map_kwargs(cfg):
    return {"prec": cfg.get("prec", BEST.get("prec", "f32r"))}


def _get_nc():
    if "nc" not in _NC_CACHE:
        _NC_CACHE["nc"] = _build(**BEST)
    return _NC_CACHE["nc"]


def _make_in_maps(x, Wq, Wk, Wv, prec="f32r"):
    qk_np = {"f32r": np.float32, "bf16": ml_dtypes.bfloat16, "fp16": np.float16}[prec]
    lp_np = {"f32r": ml_dtypes.bfloat16, "bf16": ml_dtypes.bfloat16,
             "fp16": np.float16}[prec]
    x = np.asarray(x, np.float32)
    wq8 = np.asarray(Wq, np.float32) * 0.125
    wk = np.asarray(Wk, np.float32)
    wv = np.asarray(Wv, np.float32)
    in_maps = []
    for b in range(B):
        xb = x[b]                                  # [C, N]
        qt = np.ascontiguousarray(wq8 @ xb)        # [C, N]
        kf = wk @ xb                               # [C, N]
        vt = (wv @ xb).T                           # [N, C]
        v3 = vt.reshape(NCH, P, C)
        va = np.concatenate([v3, np.ones((NCH, P, 1), np.float32)], axis=2)
        va = np.ascontiguousarray(
            va.transpose(1, 0, 2).reshape(P, NCH * CP1)
        ).astype(lp_np)
        for h in range(2):
            in_maps.append(
                {
                    "q": qt.astype(qk_np),
                    "k": np.ascontiguousarray(
                        kf[:, h * MLOC:(h + 1) * MLOC]
                    ).astype(qk_np),
                    "v": va,
                }
            )
    return in_maps


def _assemble(results):
    out = np.empty((B, C, N), np.float32)
    for core in range(2 * B):
        b, h = divmod(core, 2)
        r = results[core]["out"]
        out[b, :, h * MLOC:(h + 1) * MLOC] = r[:C] / r[C:C + 1]
    return out


def run(x, Wq, Wk, Wv, trace=False, **trace_kwargs):
    nc = _get_nc()
    res = run_bass_kernel_spmd(
        nc,
        _make_in_maps(x, Wq, Wk, Wv, prec=BEST.get("prec", "f32r")),
        core_ids=list(range(2 * B)),
        trace=trace,
        **trace_kwargs,
    )
    return _assemble(res.results), res


def kernel(x, Wq, Wk, Wv):
    out, _ = run(x, Wq, Wk, Wv, trace=False)
    return out



# revision 7
# speedup vs baseline: 1.6371x; 1.6371x over previous
"""Trainium2 Bass kernel for nn_AttentionLayer (B=4, C=64, N=4096, dk=64).

Math (per batch b):
    q_t[d, n] = (Wq/8) @ x[b]          # [64, N]
    k[d, m]   = Wk @ x[b]              # [64, N]
    v_t[n, o] = (Wv @ x[b]).T          # [N, 64]
    s[n, m]   = q_t.T @ k              # [N, N]
    attn      = softmax over n (columns)
    out[o, m] = v.T @ attn             # [64, N]

Sharding: 8 cores = 4 batches x 2 column-halves; core (b, h) computes
out[b, :, h*2048:(h+1)*2048]. The softmax axis n is fully local -> no
collectives. The tiny projections (0.25% of FLOPs) run on host so the
device inputs can be fed pre-laid-out in the matmul dtypes.

Device kernel per core, fp16 compute. The per-core floor is two engines:
  - TensorE: scores output-bound (128 PSUM writes/cyc) + AV input-bound
    (streams exp(S) once at 128/cyc) = 131072 cyc ~ 54.6us @ 2.4 GHz.
  - ScalarE: exp of 8.4M elements at 1/cyc/lane @ 1.2 GHz ~ 54.6us + the
    +352cyc/instruction overhead -> ~59-67us if ACT does all of it.
To balance, every `dve_every`-th row-chunk's exp is computed on VectorE
with a one-instruction Schraudolph approximation (bf16 bits = int16(
s*184.66 + 16249), rms rel err ~1.6% on alpha=25% of columns -> ~0.8%
output error), which pulls ACT back under the PE roofline.

Schedule (per m-tile of 512 cols, 32 row-chunks): units of 3 ACT chunks
or 1 DVE chunk; the PE stream interleaves scores-MMs of unit u with
AV-MMs of unit u-1 so it never waits on exp. PSUM: 3x2 banks scores-ACT,
1 bank scores-DVE, 1 bank AV accumulation ([65,512]; 65th v-row of ones
gives the softmax column sums). Host divides and reassembles.
"""

import ml_dtypes
import numpy as np

import concourse.bass as bass  # noqa: F401  (registers engine methods)
import concourse.mybir as mybir
import concourse.tile as tile
from concourse import bacc
from concourse.bass_utils import run_bass_kernel_spmd

B, C, N = 4, 64, 4096
MLOC = N // 2            # columns per core
P = 128
NCH = N // P             # 32 row-chunks of the score matrix
MT = 512                 # m-tile width (PSUM free dim)
NMT = MLOC // MT         # 4 m-tiles per core
CP1 = C + 1              # v columns + ones column

F32 = mybir.dt.float32
BF16 = mybir.dt.bfloat16
FP16 = mybir.dt.float16
I16 = mybir.dt.int16
EXP = mybir.ActivationFunctionType.Exp
MULT = mybir.AluOpType.mult
ADD = mybir.AluOpType.add

# Schraudolph bf16 exp: bits16(exp(s)) ~= int16(s*SCHA + SCHB) (RNE cvt).
# c* = -0.054585 minimizes rms multiplicative error (~1.54%).
SCHA = 128.0 / float(np.log(2.0))            # 184.6650
SCHB = 16256.0 + 128.0 * (-0.054585)         # 16249.013

_NC_CACHE = {}


def _schedule(dve_every, ga=3):
    """One m-tile's unit list: ('act', [chunks...]) / ('dve', [chunk])."""
    units = []
    run = []

    def flush():
        while run:
            # split the ACT run into groups of <= ga, as evenly as possible
            ngrp = -(-len(run) // ga)
            take = -(-len(run) // ngrp)
            units.append(("act", run[:take]))
            del run[:take]

    for i in range(NCH):
        if dve_every and (i % dve_every == dve_every - 1):
            flush()
            units.append(("dve", [i]))
        else:
            run.append(i)
    flush()
    return units


def _build(dve_every=4, ga=3, spsum_bufs=2, exp_bufs=2, apsum_bufs=None,
           loop_reps=None):
    """Build the per-core graph.

    dve_every: every k-th row-chunk's exp runs on VectorE (Schraudolph
        bf16); 0 = all chunks on ScalarE.
    ga: ACT group size (chunks exp'd per ScalarE instruction).
    loop_reps: wrap the attention body in a hardware For_i loop (timing
        only: per-iteration time = loop-slope over two rep counts).
    """
    units = _schedule(dve_every, ga)
    n_dve = sum(1 for k, _ in units if k == "dve")
    if apsum_bufs is None:
        apsum_bufs = 1 if n_dve else 2
    assert ga * spsum_bufs + (1 if n_dve else 0) + apsum_bufs <= 8

    nc = bacc.Bacc("TRN2", target_bir_lowering=False, debug=False)
    q_ext = nc.declare_dram_parameter("q", [C, N], FP16, isOutput=False)
    k_ext = nc.declare_dram_parameter("k", [C, MLOC], FP16, isOutput=False)
    v_ext = nc.declare_dram_parameter("v", [P, NCH * CP1], FP16, isOutput=False)
    if n_dve:
        vb_ext = nc.declare_dram_parameter(
            "vb", [P, NCH * CP1], BF16, isOutput=False)
    out_ext = nc.declare_dram_parameter("out", [CP1, MLOC], F32, isOutput=True)

    with tile.TileContext(nc) as tc:
        with (
            tc.tile_pool(name="const", bufs=1) as cpool,
            tc.tile_pool(name="expp", bufs=exp_bufs) as epool,
            tc.tile_pool(name="outp", bufs=2) as opool,
            tc.tile_pool(name="spsum", bufs=spsum_bufs, space="PSUM") as spool,
            tc.tile_pool(name="dpsum", bufs=1, space="PSUM") as dpool,
            tc.tile_pool(name="apsum", bufs=apsum_bufs, space="PSUM") as apool,
        ):
            # One serial HWDGE queue, emitted in first-needed order; v/vb
            # stream on the SWDGE (gpsimd) queue in parallel.
            k_sb = cpool.tile([C, MLOC], FP16)
            q_sb = cpool.tile([C, N], FP16)
            v_sb = cpool.tile([P, NCH * CP1], FP16)
            vb_sb = (cpool.tile([P, NCH * CP1], BF16, name="vb_sb")
                     if n_dve else None)
            vw = NCH * CP1 // 4

            def dq(j, eng=nc.sync):
                eng.dma_start(
                    q_sb[:, j * 512:(j + 1) * 512], q_ext[:, j * 512:(j + 1) * 512]
                )

            def dk(j, eng=nc.sync):
                eng.dma_start(
                    k_sb[:, j * 512:(j + 1) * 512], k_ext[:, j * 512:(j + 1) * 512]
                )

            dq(0); dk(0); dq(1); dq(2); dq(3); dq(4); dq(5); dq(6); dq(7)
            dk(1); dk(2); dk(3)
            for j in range(4):
                nc.gpsimd.dma_start(
                    v_sb[:, j * vw:(j + 1) * vw], v_ext[:, j * vw:(j + 1) * vw])
                if n_dve:
                    nc.gpsimd.dma_start(
                        vb_sb[:, j * vw:(j + 1) * vw], vb_ext[:, j * vw:(j + 1) * vw])

            def q_ap(i):
                return q_sb[:, i * P:(i + 1) * P]

            def k_ap(t):
                return k_sb[:, t * MT:(t + 1) * MT]

            def attention_body(iv=None):
                flat = [(t, kind, chs) for t in range(NMT)
                        for kind, chs in units]
                exp_tiles, pav_tiles = {}, {}
                prev = None  # (t, chunks) awaiting AV emission

                def emit_av(t, chunks):
                    if t not in pav_tiles:
                        pav_tiles[t] = apool.tile(
                            [CP1, MT], F32, tag="av", name="pav")
                    pav = pav_tiles[t]
                    exp_sb = exp_tiles[t]
                    for i in chunks:
                        dve = dve_every and (i % dve_every == dve_every - 1)
                        rhs = exp_sb[:, i * MT:(i + 1) * MT]
                        nc.tensor.matmul(
                            pav[:],
                            lhsT=(vb_sb if dve else v_sb)[:, i * CP1:(i + 1) * CP1],
                            rhs=rhs.bitcast(BF16) if dve else rhs,
                            start=(i == 0),
                            stop=(i == NCH - 1),
                        )
                    if chunks[-1] == NCH - 1:
                        o_sb = opool.tile([CP1, MT], F32, tag="ot")
                        nc.vector.tensor_copy(o_sb[:], pav[:])
                        nc.sync.dma_start(out_ext[:, t * MT:(t + 1) * MT], o_sb[:])

                for t, kind, chunks in flat:
                    if t not in exp_tiles:
                        exp_tiles[t] = epool.tile(
                            [P, NCH * MT], FP16, tag="exp", name="exp_sb")
                    exp_sb = exp_tiles[t]
                    if kind == "act":
                        g = len(chunks)
                        ps = spool.tile([P, ga * MT], F32, tag="sc")
                        for u, i in enumerate(chunks):
                            nc.tensor.matmul(
                                ps[:, u * MT:(u + 1) * MT],
                                lhsT=q_ap(i), rhs=k_ap(t),
                                start=True, stop=True,
                            )
                        nc.scalar.activation(
                            exp_sb[:, chunks[0] * MT:(chunks[-1] + 1) * MT],
                            ps[:, :g * MT], EXP,
                        )
                    else:
                        i = chunks[0]
                        ps = dpool.tile([P, MT], F32, tag="dv")
                        nc.tensor.matmul(
                            ps[:], lhsT=q_ap(i), rhs=k_ap(t),
                            start=True, stop=True,
                        )
                        nc.vector.tensor_scalar(
                            exp_sb[:, i * MT:(i + 1) * MT].bitcast(I16),
                            ps[:], SCHA, SCHB, MULT, ADD,
                        )
                    if prev is not None:
                        emit_av(*prev)
                    prev = (t, chunks)
                emit_av(*prev)

            if loop_reps is None:
                attention_body()
            else:
                with tc.For_i(0, loop_reps, 1):
                    attention_body()

    nc.compile()
    return nc


BEST = {"dve_every": 4, "ga": 3}


def _in_map_kwargs(cfg):
    return {"with_vb": bool(cfg.get("dve_every", BEST.get("dve_every")))}


def _get_nc():
    if "nc" not in _NC_CACHE:
        _NC_CACHE["nc"] = _build(**BEST)
    return _NC_CACHE["nc"]


def _make_in_maps(x, Wq, Wk, Wv, with_vb=True):
    x = np.asarray(x, np.float32)
    wq8 = np.asarray(Wq, np.float32) * 0.125
    wk = np.asarray(Wk, np.float32)
    wv = np.asarray(Wv, np.float32)
    in_maps = []
    for b in range(B):
        xb = x[b]                                  # [C, N]
        qt = np.ascontiguousarray(wq8 @ xb)        # [C, N]
        kf = wk @ xb                               # [C, N]
        vt = (wv @ xb).T                           # [N, C]
        v3 = vt.reshape(NCH, P, C)
        va = np.concatenate([v3, np.ones((NCH, P, 1), np.float32)], axis=2)
        va = np.ascontiguousarray(va.transpose(1, 0, 2).reshape(P, NCH * CP1))
        for h in range(2):
            m = {
                "q": qt.astype(np.float16),
                "k": np.ascontiguousarray(
                    kf[:, h * MLOC:(h + 1) * MLOC]
                ).astype(np.float16),
                "v": va.astype(np.float16),
            }
            if with_vb:
                m["vb"] = va.astype(ml_dtypes.bfloat16)
            in_maps.append(m)
    return in_maps


def _assemble(results):
    out = np.empty((B, C, N), np.float32)
    for core in range(2 * B):
        b, h = divmod(core, 2)
        r = results[core]["out"]
        out[b, :, h * MLOC:(h + 1) * MLOC] = r[:C] / r[C:C + 1]
    return out


def run(x, Wq, Wk, Wv, trace=False, **trace_kwargs):
    nc = _get_nc()
    res = run_bass_kernel_spmd(
        nc,
        _make_in_maps(x, Wq, Wk, Wv, **_in_map_kwargs(BEST)),
        core_ids=list(range(2 * B)),
        trace=trace,
        **trace_kwargs,
    )
    return _assemble(res.results), res


def kernel(x, Wq, Wk, Wv):
    out, _ = run(x, Wq, Wk, Wv, trace=False)
    return out


# revision 26
# speedup vs baseline: 2.0711x; 1.2651x over previous
"""Trainium2 Bass kernel for nn_AttentionLayer (B=4, C=64, N=4096, dk=64).

Math (per batch b):
    q_t[d, n] = (Wq/8) @ x[b]          # [64, N]
    k[d, m]   = Wk @ x[b]              # [64, N]
    v_t[n, o] = (Wv @ x[b]).T          # [N, 64]
    s[n, m]   = q_t.T @ k              # [N, N]
    attn      = softmax over n (columns)
    out[o, m] = v.T @ attn             # [64, N]

Sharding: 8 cores = 4 batches x 2 column-halves; core (b, h) computes
out[b, :, h*2048:(h+1)*2048]. The softmax axis n is fully local -> no
collectives. The tiny projections (0.25% of FLOPs) run on host so the
device inputs can be fed pre-laid-out in the matmul dtypes.

Device kernel per core, fp16 compute. Measured reality on this hw (not
what the cost model says):
  - ACT exp from PSUM: ~653 ns per [128,512] chunk (~1.5 cyc/elem, the
    trn2 SBUF/PSUM errata). The all-ACT baseline was exactly
    ACT-saturated: 128 chunks x 653ns = 83.6us.
  - DVE approximate exp in one tensor_scalar (Schraudolph: bf16 bits =
    int16(s*184.66 + 16249), rms rel err ~1.6%) at ~1.5us per chunk
    when done in 2-chunk [128,1024] instructions; routing 8 of 32
    chunks per tile ('p8') to DVE de-saturates ACT (output rel err
    1.7e-3, well under the 2e-2 gate).
  - Accumulating matmuls write PSUM at 2 cyc/column (RMW), so the AV
    burst costs 427ns/chunk vs scores 216ns/chunk. With exp off the
    critical path the kernel sits exactly on this PE floor:
    128 x (216 + 427) ns = 82.3us  (measured 82.1us loop-slope).
  - Variants tried and measured slower: hand-interleaved scores/AV
    emission with AV lag (104-122us; the Tile scheduler's own ordering
    of the phase-separated emission wins), flipped AV with E as
    LDWEIGHTS stationary operand (93us; weight-load bandwidth-bound
    in-context), fp8/DoubleRow (fails the error budget on v).

PSUM: 3x2 banks scores groups (ACT groups of 3 and DVE pairs share the
pool), 2 banks AV accumulation ([65,512]; the 65th v-row of ones gives
the softmax column sums). Host divides numerator by colsum and
reassembles.
"""

import ml_dtypes
import numpy as np

import concourse.bass as bass  # noqa: F401  (registers engine methods)
import concourse.mybir as mybir
import concourse.tile as tile
from concourse import bacc
from concourse.bass_utils import run_bass_kernel_spmd

B, C, N = 4, 64, 4096
MLOC = N // 2            # columns per core
P = 128
NCH = N // P             # 32 row-chunks of the score matrix
MT = 512                 # m-tile width (PSUM free dim)
NMT = MLOC // MT         # 4 m-tiles per core
CP1 = C + 1              # v columns + ones column

F32 = mybir.dt.float32
BF16 = mybir.dt.bfloat16
FP16 = mybir.dt.float16
I16 = mybir.dt.int16
EXP = mybir.ActivationFunctionType.Exp
MULT = mybir.AluOpType.mult
ADD = mybir.AluOpType.add

# Schraudolph bf16 exp: bits16(exp(s)) ~= int16(s*SCHA + SCHB) (RNE cvt).
# c* = -0.054585 minimizes rms multiplicative error (~1.54%).
SCHA = 128.0 / float(np.log(2.0))            # 184.6650
SCHB = 16256.0 + 128.0 * (-0.054585)         # 16249.013

_NC_CACHE = {}


def _dve_set(pat):
    """Chunk indices whose exp runs on VectorE. Patterns: '' none,
    'sK' singles every K chunks, 'pK' adjacent pairs per K-octet,
    'tK' adjacent triples per K-octet, 'x8' alternating pairs/triples."""
    if not pat:
        return frozenset()
    if pat == "x8":
        return frozenset(
            i for i in range(NCH) if i % 8 >= (6 if (i // 8) % 2 == 0 else 5))
    kind, k = pat[0], int(pat[1:])
    n = {"s": 1, "p": 2, "t": 3}[kind]
    return frozenset(i for i in range(NCH) if i % k >= k - n)


def _schedule(dve_set, ga=3):
    """One m-tile's unit list: ('act'|'dve', [chunks...]). Consecutive
    same-engine chunks group into units of <= ga."""
    units = []
    run = []
    run_kind = None

    def flush():
        while run:
            ngrp = -(-len(run) // ga)
            take = -(-len(run) // ngrp)
            units.append((run_kind, run[:take]))
            del run[:take]

    for i in range(NCH):
        kind = "dve" if i in dve_set else "act"
        if kind != run_kind:
            flush()
            run_kind = kind
        run.append(i)
    flush()
    return units


def _build(dve_pat="p8", ga=3, spsum_bufs=2, exp_bufs=2, apsum_bufs=None,
           flip_av=False, loop_reps=None):
    """Build the per-core graph.

    dve_pat: which row-chunks' exp runs on VectorE (Schraudolph bf16)
        instead of ScalarE: '' none, 's4' singles every 4, 'p8' pairs
        per octet, 't8' triples per octet. Multi-chunk patterns amortize
        the ~1.5us fixed cost of a DVE PSUM read.
    ga: ACT group size (chunks exp'd per ScalarE instruction).
    flip_av: accumulate AV with E as the stationary operand (LDWEIGHTS)
        and v as the 65-column moving operand, so the accumulating PSUM
        writes (2 cyc/col RMW) drop from 512 to 4x65 columns per chunk.
        Output becomes [128 m-rows, 4 m-blocks x 65] per tile,
        transposed on host.
    loop_reps: wrap the attention body in a hardware For_i loop (timing
        only: per-iteration time = loop-slope over two rep counts).
    """
    dve_set = _dve_set(dve_pat)
    units = _schedule(dve_set, ga)
    n_act = sum(1 for k, _ in units if k == "act")
    n_dve = sum(1 for k, _ in units if k == "dve")
    # Multi-chunk DVE units allocate their scores PSUM from the shared
    # spool (one pool rotation, no extra banks); single-chunk DVE units
    # get a dedicated 1-bank pool so they don't burn a 3-bank slot.
    dve_share_spool = n_dve > 0 and all(
        len(c) >= 2 for k, c in units if k == "dve")
    dpool_bufs = 0 if (dve_share_spool or not n_dve) else (1 if n_act else 6)
    if apsum_bufs is None:
        apsum_bufs = 2 if dpool_bufs == 0 else 1
    assert (ga * spsum_bufs if (n_act or dve_share_spool) else 0) + \
        dpool_bufs + apsum_bufs <= 8

    nc = bacc.Bacc("TRN2", target_bir_lowering=False, debug=False)
    q_ext = nc.declare_dram_parameter("q", [C, N], FP16, isOutput=False)
    k_ext = nc.declare_dram_parameter("k", [C, MLOC], FP16, isOutput=False)
    v_ext = nc.declare_dram_parameter("v", [P, NCH * CP1], FP16, isOutput=False)
    if n_dve:
        vb_ext = nc.declare_dram_parameter(
            "vb", [P, NCH * CP1], BF16, isOutput=False)
    out_shape = [P, NMT * 4 * CP1] if flip_av else [CP1, MLOC]
    out_ext = nc.declare_dram_parameter("out", out_shape, F32, isOutput=True)

    with tile.TileContext(nc) as tc:
        with (
            tc.tile_pool(name="const", bufs=1) as cpool,
            tc.tile_pool(name="expp", bufs=exp_bufs) as epool,
            tc.tile_pool(name="outp", bufs=2) as opool,
            tc.tile_pool(name="spsum", bufs=spsum_bufs, space="PSUM") as spool,
            tc.tile_pool(name="dpsum", bufs=max(dpool_bufs, 1),
                         space="PSUM") as dpool,
            tc.tile_pool(name="apsum", bufs=apsum_bufs, space="PSUM") as apool,
        ):
            # One serial HWDGE queue, emitted in first-needed order; v/vb
            # stream on the SWDGE (gpsimd) queue in parallel.
            k_sb = cpool.tile([C, MLOC], FP16)
            q_sb = cpool.tile([C, N], FP16)
            v_sb = cpool.tile([P, NCH * CP1], FP16)
            vb_sb = (cpool.tile([P, NCH * CP1], BF16, name="vb_sb")
                     if n_dve else None)
            vw = NCH * CP1 // 4

            def dq(j, eng=nc.sync):
                eng.dma_start(
                    q_sb[:, j * 512:(j + 1) * 512], q_ext[:, j * 512:(j + 1) * 512]
                )

            def dk(j, eng=nc.sync):
                eng.dma_start(
                    k_sb[:, j * 512:(j + 1) * 512], k_ext[:, j * 512:(j + 1) * 512]
                )

            dq(0); dk(0); dq(1); dq(2); dq(3); dq(4); dq(5); dq(6); dq(7)
            dk(1); dk(2); dk(3)
            for j in range(4):
                nc.gpsimd.dma_start(
                    v_sb[:, j * vw:(j + 1) * vw], v_ext[:, j * vw:(j + 1) * vw])
                if n_dve:
                    nc.gpsimd.dma_start(
                        vb_sb[:, j * vw:(j + 1) * vw], vb_ext[:, j * vw:(j + 1) * vw])

            def q_ap(i):
                return q_sb[:, i * P:(i + 1) * P]

            def k_ap(t):
                return k_sb[:, t * MT:(t + 1) * MT]

            def attention_body(iv=None):
                for t in range(NMT):
                    exp_sb = epool.tile([P, NCH * MT], FP16, tag="exp",
                                        name="exp_sb")
                    for kind, chunks in units:
                        g = len(chunks)
                        if kind == "act" or dve_share_spool:
                            ps = spool.tile([P, ga * MT], F32, tag="sc",
                                            name="ps")
                        else:
                            ps = dpool.tile([P, MT], F32, tag="dv", name="ps")
                        for u, i in enumerate(chunks):
                            nc.tensor.matmul(
                                ps[:, u * MT:(u + 1) * MT],
                                lhsT=q_ap(i), rhs=k_ap(t),
                                start=True, stop=True,
                            )
                        dst = exp_sb[:, chunks[0] * MT:(chunks[-1] + 1) * MT]
                        if kind == "act":
                            nc.scalar.activation(dst, ps[:, :g * MT], EXP)
                        else:
                            nc.vector.tensor_scalar(
                                dst.bitcast(I16), ps[:, :g * MT],
                                SCHA, SCHB, MULT, ADD,
                            )
                    if flip_av:
                        w = 4 * CP1
                        pav = apool.tile([P, w], F32, tag="av", name="pav")
                        for i in range(NCH):
                            dve = i in dve_set
                            vs = (vb_sb if dve else v_sb)[:, i * CP1:(i + 1) * CP1]
                            for mb in range(4):
                                eb = exp_sb[:, i * MT + mb * P:i * MT + (mb + 1) * P]
                                # start=True zeroes the WHOLE psum bank, so
                                # only the very first MM of the bank starts
                                # (the bank-clear leaves has_written=0 for the
                                # other mb regions -> accumulate onto zero).
                                nc.tensor.matmul(
                                    pav[:, mb * CP1:(mb + 1) * CP1],
                                    lhsT=eb.bitcast(BF16) if dve else eb,
                                    rhs=vs,
                                    start=(i == 0 and mb == 0),
                                    stop=(i == NCH - 1 and mb == 3),
                                    skip_group_check=True,
                                )
                        o_sb = opool.tile([P, w], F32, tag="ot", name="o_sb")
                        nc.vector.tensor_copy(o_sb[:], pav[:])
                        nc.sync.dma_start(out_ext[:, t * w:(t + 1) * w], o_sb[:])
                    else:
                        pav = apool.tile([CP1, MT], F32, tag="av", name="pav")
                        for i in range(NCH):
                            dve = i in dve_set
                            rhs = exp_sb[:, i * MT:(i + 1) * MT]
                            nc.tensor.matmul(
                                pav[:],
                                lhsT=(vb_sb if dve else v_sb)[:, i * CP1:(i + 1) * CP1],
                                rhs=rhs.bitcast(BF16) if dve else rhs,
                                start=(i == 0),
                                stop=(i == NCH - 1),
                            )
                        o_sb = opool.tile([CP1, MT], F32, tag="ot", name="o_sb")
                        nc.vector.tensor_copy(o_sb[:], pav[:])
                        nc.sync.dma_start(out_ext[:, t * MT:(t + 1) * MT], o_sb[:])

            if loop_reps is None:
                attention_body()
            else:
                with tc.For_i(0, loop_reps, 1):
                    attention_body()

    nc.compile()
    return nc


BEST = {"dve_pat": "p8", "ga": 3}


def _in_map_kwargs(cfg):
    return {"with_vb": bool(cfg.get("dve_pat", BEST.get("dve_pat")))}


def _get_nc():
    if "nc" not in _NC_CACHE:
        _NC_CACHE["nc"] = _build(**BEST)
    return _NC_CACHE["nc"]


def _make_in_maps(x, Wq, Wk, Wv, with_vb=True):
    x = np.asarray(x, np.float32)
    wq8 = np.asarray(Wq, np.float32) * 0.125
    wk = np.asarray(Wk, np.float32)
    wv = np.asarray(Wv, np.float32)
    in_maps = []
    for b in range(B):
        xb = x[b]                                  # [C, N]
        qt = np.ascontiguousarray(wq8 @ xb)        # [C, N]
        kf = wk @ xb                               # [C, N]
        vt = (wv @ xb).T                           # [N, C]
        v3 = vt.reshape(NCH, P, C)
        va = np.concatenate([v3, np.ones((NCH, P, 1), np.float32)], axis=2)
        va = np.ascontiguousarray(va.transpose(1, 0, 2).reshape(P, NCH * CP1))
        for h in range(2):
            m = {
                "q": qt.astype(np.float16),
                "k": np.ascontiguousarray(
                    kf[:, h * MLOC:(h + 1) * MLOC]
                ).astype(np.float16),
                "v": va.astype(np.float16),
            }
            if with_vb:
                m["vb"] = va.astype(ml_dtypes.bfloat16)
            in_maps.append(m)
    return in_maps


def _assemble(results):
    out = np.empty((B, C, N), np.float32)
    for core in range(2 * B):
        b, h = divmod(core, 2)
        r = results[core]["out"]
        if r.shape == (CP1, MLOC):
            half = r[:C] / r[C:C + 1]
        else:  # flip_av: [128 m-rows, NMT * 4 * 65]
            rr = r.reshape(P, NMT, 4, CP1)
            vals = rr[:, :, :, :C] / rr[:, :, :, C:]     # [mrow, t, mb, o]
            half = np.transpose(vals, (3, 1, 2, 0)).reshape(C, MLOC)
        out[b, :, h * MLOC:(h + 1) * MLOC] = half
    return out


def run(x, Wq, Wk, Wv, trace=False, **trace_kwargs):
    nc = _get_nc()
    res = run_bass_kernel_spmd(
        nc,
        _make_in_maps(x, Wq, Wk, Wv, **_in_map_kwargs(BEST)),
        core_ids=list(range(2 * B)),
        trace=trace,
        **trace_kwargs,
    )
    return _assemble(res.results), res


def kernel(x, Wq, Wk, Wv):
    out, _ = run(x, Wq, Wk, Wv, trace=False)
    return out


# revision 29
# speedup vs baseline: 2.2566x; 1.0896x over previous
"""Trainium2 Bass kernel for nn_AttentionLayer (B=4, C=64, N=4096, dk=64).

Math (per batch b):
    q_t[d, n] = (Wq/8) @ x[b]          # [64, N]
    k[d, m]   = Wk @ x[b]              # [64, N]
    v_t[n, o] = (Wv @ x[b]).T          # [N, 64]
    s[n, m]   = q_t.T @ k              # [N, N]
    attn      = softmax over n (columns)
    out[o, m] = v.T @ attn             # [64, N]

Sharding: 8 cores = 4 batches x 2 column-halves; core (b, h) computes
out[b, :, h*2048:(h+1)*2048]. The softmax axis n is fully local -> no
collectives. The tiny projections (0.25% of FLOPs) run on host so the
device inputs can be fed pre-laid-out in the matmul dtypes.

Device kernel per core, fp16 compute. Measured reality on this hw (not
what the cost model says):
  - ACT exp from PSUM: ~653 ns per [128,512] chunk (~1.5 cyc/elem, the
    trn2 SBUF/PSUM errata). The all-ACT baseline was exactly
    ACT-saturated: 128 chunks x 653ns = 83.6us.
  - DVE approximate exp in one tensor_scalar (Schraudolph: bf16 bits =
    int16(s*184.66 + 16249), rms rel err ~1.6%) at ~1.5us per chunk
    when done in multi-chunk [128,1024]/[128,1536] instructions;
    routing 10 of 32 chunks per tile ('x8': alternating pairs/triples
    per octet) to DVE de-saturates ACT (output rel err 1.9e-3, well
    under the 2e-2 gate). Same-session A/B: x8 80.2us vs p8 83.5us vs
    all-ACT 102us.
  - Accumulating matmuls write PSUM at 2 cyc/column (RMW), so the AV
    burst costs 427ns/chunk vs scores 216ns/chunk. With exp off the
    critical path the kernel sits exactly on this PE floor:
    128 x (216 + 427) ns = 82.3us  (measured 82.1us loop-slope).
  - Variants tried and measured slower: hand-interleaved scores/AV
    emission with AV lag (104-122us; the Tile scheduler's own ordering
    of the phase-separated emission wins), flipped AV with E as
    LDWEIGHTS stationary operand (93us; weight-load bandwidth-bound
    in-context), fp8/DoubleRow (fails the error budget on v).

PSUM: 3x2 banks scores groups (ACT groups of 3 and DVE pairs share the
pool), 2 banks AV accumulation ([65,512]; the 65th v-row of ones gives
the softmax column sums). Host divides numerator by colsum and
reassembles.
"""

import ml_dtypes
import numpy as np

import concourse.bass as bass  # noqa: F401  (registers engine methods)
import concourse.mybir as mybir
import concourse.tile as tile
from concourse import bacc
from concourse.bass_utils import run_bass_kernel_spmd

B, C, N = 4, 64, 4096
MLOC = N // 2            # columns per core
P = 128
NCH = N // P             # 32 row-chunks of the score matrix
MT = 512                 # m-tile width (PSUM free dim)
NMT = MLOC // MT         # 4 m-tiles per core
CP1 = C + 1              # v columns + ones column

F32 = mybir.dt.float32
BF16 = mybir.dt.bfloat16
FP16 = mybir.dt.float16
I16 = mybir.dt.int16
EXP = mybir.ActivationFunctionType.Exp
MULT = mybir.AluOpType.mult
ADD = mybir.AluOpType.add

# Schraudolph bf16 exp: bits16(exp(s)) ~= int16(s*SCHA + SCHB) (RNE cvt).
# c* = -0.054585 minimizes rms multiplicative error (~1.54%).
SCHA = 128.0 / float(np.log(2.0))            # 184.6650
SCHB = 16256.0 + 128.0 * (-0.054585)         # 16249.013

_NC_CACHE = {}


def _dve_set(pat):
    """Chunk indices whose exp runs on VectorE. Patterns: '' none,
    'sK' singles every K chunks, 'pK' adjacent pairs per K-octet,
    'tK' adjacent triples per K-octet, 'x8' alternating pairs/triples."""
    if not pat:
        return frozenset()
    if pat == "x8":
        return frozenset(
            i for i in range(NCH) if i % 8 >= (6 if (i // 8) % 2 == 0 else 5))
    if pat[0] == "e":  # last K chunks of the tile
        return frozenset(range(NCH - int(pat[1:]), NCH))
    kind, k = pat[0], int(pat[1:])
    n = {"s": 1, "p": 2, "t": 3}[kind]
    return frozenset(i for i in range(NCH) if i % k >= k - n)


def _schedule(dve_set, ga=3):
    """One m-tile's unit list: ('act'|'dve', [chunks...]). Consecutive
    same-engine chunks group into units of <= ga."""
    units = []
    run = []
    run_kind = None

    def flush():
        while run:
            ngrp = -(-len(run) // ga)
            take = -(-len(run) // ngrp)
            units.append((run_kind, run[:take]))
            del run[:take]

    for i in range(NCH):
        kind = "dve" if i in dve_set else "act"
        if kind != run_kind:
            flush()
            run_kind = kind
        run.append(i)
    flush()
    return units


def _build(dve_pat="p8", ga=3, spsum_bufs=2, exp_bufs=2, apsum_bufs=None,
           flip_av=False, loop_reps=None):
    """Build the per-core graph.

    dve_pat: which row-chunks' exp runs on VectorE (Schraudolph bf16)
        instead of ScalarE: '' none, 's4' singles every 4, 'p8' pairs
        per octet, 't8' triples per octet. Multi-chunk patterns amortize
        the ~1.5us fixed cost of a DVE PSUM read.
    ga: ACT group size (chunks exp'd per ScalarE instruction).
    flip_av: accumulate AV with E as the stationary operand (LDWEIGHTS)
        and v as the 65-column moving operand, so the accumulating PSUM
        writes (2 cyc/col RMW) drop from 512 to 4x65 columns per chunk.
        Output becomes [128 m-rows, 4 m-blocks x 65] per tile,
        transposed on host.
    loop_reps: wrap the attention body in a hardware For_i loop (timing
        only: per-iteration time = loop-slope over two rep counts).
    """
    dve_set = _dve_set(dve_pat)
    units = _schedule(dve_set, ga)
    n_act = sum(1 for k, _ in units if k == "act")
    n_dve = sum(1 for k, _ in units if k == "dve")
    # Multi-chunk DVE units allocate their scores PSUM from the shared
    # spool (one pool rotation, no extra banks); single-chunk DVE units
    # get a dedicated 1-bank pool so they don't burn a 3-bank slot.
    dve_share_spool = n_dve > 0 and all(
        len(c) >= 2 for k, c in units if k == "dve")
    dpool_bufs = 0 if (dve_share_spool or not n_dve) else (1 if n_act else 6)
    if apsum_bufs is None:
        apsum_bufs = 2 if dpool_bufs == 0 else 1
    assert (ga * spsum_bufs if (n_act or dve_share_spool) else 0) + \
        dpool_bufs + apsum_bufs <= 8

    nc = bacc.Bacc("TRN2", target_bir_lowering=False, debug=False)
    q_ext = nc.declare_dram_parameter("q", [C, N], FP16, isOutput=False)
    k_ext = nc.declare_dram_parameter("k", [C, MLOC], FP16, isOutput=False)
    v_ext = nc.declare_dram_parameter("v", [P, NCH * CP1], FP16, isOutput=False)
    if n_dve:
        vb_ext = nc.declare_dram_parameter(
            "vb", [P, NCH * CP1], BF16, isOutput=False)
    out_shape = [P, NMT * 4 * CP1] if flip_av else [CP1, MLOC]
    out_ext = nc.declare_dram_parameter("out", out_shape, F32, isOutput=True)

    with tile.TileContext(nc) as tc:
        with (
            tc.tile_pool(name="const", bufs=1) as cpool,
            tc.tile_pool(name="expp", bufs=exp_bufs) as epool,
            tc.tile_pool(name="outp", bufs=2) as opool,
            tc.tile_pool(name="spsum", bufs=spsum_bufs, space="PSUM") as spool,
            tc.tile_pool(name="dpsum", bufs=max(dpool_bufs, 1),
                         space="PSUM") as dpool,
            tc.tile_pool(name="apsum", bufs=apsum_bufs, space="PSUM") as apool,
        ):
            # One serial HWDGE queue, emitted in first-needed order; v/vb
            # stream on the SWDGE (gpsimd) queue in parallel.
            k_sb = cpool.tile([C, MLOC], FP16)
            q_sb = cpool.tile([C, N], FP16)
            v_sb = cpool.tile([P, NCH * CP1], FP16)
            vb_sb = (cpool.tile([P, NCH * CP1], BF16, name="vb_sb")
                     if n_dve else None)
            vw = NCH * CP1 // 4

            def dq(j, eng=nc.sync):
                eng.dma_start(
                    q_sb[:, j * 512:(j + 1) * 512], q_ext[:, j * 512:(j + 1) * 512]
                )

            def dk(j, eng=nc.sync):
                eng.dma_start(
                    k_sb[:, j * 512:(j + 1) * 512], k_ext[:, j * 512:(j + 1) * 512]
                )

            dq(0); dk(0); dq(1); dq(2); dq(3); dq(4); dq(5); dq(6); dq(7)
            dk(1); dk(2); dk(3)
            for j in range(4):
                nc.gpsimd.dma_start(
                    v_sb[:, j * vw:(j + 1) * vw], v_ext[:, j * vw:(j + 1) * vw])
                if n_dve:
                    nc.gpsimd.dma_start(
                        vb_sb[:, j * vw:(j + 1) * vw], vb_ext[:, j * vw:(j + 1) * vw])

            def q_ap(i):
                return q_sb[:, i * P:(i + 1) * P]

            def k_ap(t):
                return k_sb[:, t * MT:(t + 1) * MT]

            def attention_body(iv=None):
                for t in range(NMT):
                    exp_sb = epool.tile([P, NCH * MT], FP16, tag="exp",
                                        name="exp_sb")
                    for kind, chunks in units:
                        g = len(chunks)
                        if kind == "act" or dve_share_spool:
                            ps = spool.tile([P, ga * MT], F32, tag="sc",
                                            name="ps")
                        else:
                            ps = dpool.tile([P, MT], F32, tag="dv", name="ps")
                        for u, i in enumerate(chunks):
                            nc.tensor.matmul(
                                ps[:, u * MT:(u + 1) * MT],
                                lhsT=q_ap(i), rhs=k_ap(t),
                                start=True, stop=True,
                            )
                        dst = exp_sb[:, chunks[0] * MT:(chunks[-1] + 1) * MT]
                        if kind == "act":
                            nc.scalar.activation(dst, ps[:, :g * MT], EXP)
                        else:
                            nc.vector.tensor_scalar(
                                dst.bitcast(I16), ps[:, :g * MT],
                                SCHA, SCHB, MULT, ADD,
                            )
                    if flip_av:
                        w = 4 * CP1
                        pav = apool.tile([P, w], F32, tag="av", name="pav")
                        for i in range(NCH):
                            dve = i in dve_set
                            vs = (vb_sb if dve else v_sb)[:, i * CP1:(i + 1) * CP1]
                            for mb in range(4):
                                eb = exp_sb[:, i * MT + mb * P:i * MT + (mb + 1) * P]
                                # start=True zeroes the WHOLE psum bank, so
                                # only the very first MM of the bank starts
                                # (the bank-clear leaves has_written=0 for the
                                # other mb regions -> accumulate onto zero).
                                nc.tensor.matmul(
                                    pav[:, mb * CP1:(mb + 1) * CP1],
                                    lhsT=eb.bitcast(BF16) if dve else eb,
                                    rhs=vs,
                                    start=(i == 0 and mb == 0),
                                    stop=(i == NCH - 1 and mb == 3),
                                    skip_group_check=True,
                                )
                        o_sb = opool.tile([P, w], F32, tag="ot", name="o_sb")
                        nc.vector.tensor_copy(o_sb[:], pav[:])
                        nc.sync.dma_start(out_ext[:, t * w:(t + 1) * w], o_sb[:])
                    else:
                        pav = apool.tile([CP1, MT], F32, tag="av", name="pav")
                        for i in range(NCH):
                            dve = i in dve_set
                            rhs = exp_sb[:, i * MT:(i + 1) * MT]
                            nc.tensor.matmul(
                                pav[:],
                                lhsT=(vb_sb if dve else v_sb)[:, i * CP1:(i + 1) * CP1],
                                rhs=rhs.bitcast(BF16) if dve else rhs,
                                start=(i == 0),
                                stop=(i == NCH - 1),
                            )
                        o_sb = opool.tile([CP1, MT], F32, tag="ot", name="o_sb")
                        nc.vector.tensor_copy(o_sb[:], pav[:])
                        nc.sync.dma_start(out_ext[:, t * MT:(t + 1) * MT], o_sb[:])

            if loop_reps is None:
                attention_body()
            else:
                with tc.For_i(0, loop_reps, 1):
                    attention_body()

    nc.compile()
    return nc


BEST = {"dve_pat": "x8", "ga": 3}


def _in_map_kwargs(cfg):
    return {"with_vb": bool(cfg.get("dve_pat", BEST.get("dve_pat")))}


def _get_nc():
    if "nc" not in _NC_CACHE:
        _NC_CACHE["nc"] = _build(**BEST)
    return _NC_CACHE["nc"]


def _make_in_maps(x, Wq, Wk, Wv, with_vb=True):
    x = np.asarray(x, np.float32)
    wq8 = np.asarray(Wq, np.float32) * 0.125
    wk = np.asarray(Wk, np.float32)
    wv = np.asarray(Wv, np.float32)
    in_maps = []
    for b in range(B):
        xb = x[b]                                  # [C, N]
        qt = np.ascontiguousarray(wq8 @ xb)        # [C, N]
        kf = wk @ xb                               # [C, N]
        vt = (wv @ xb).T                           # [N, C]
        v3 = vt.reshape(NCH, P, C)
        va = np.concatenate([v3, np.ones((NCH, P, 1), np.float32)], axis=2)
        va = np.ascontiguousarray(va.transpose(1, 0, 2).reshape(P, NCH * CP1))
        for h in range(2):
            m = {
                "q": qt.astype(np.float16),
                "k": np.ascontiguousarray(
                    kf[:, h * MLOC:(h + 1) * MLOC]
                ).astype(np.float16),
                "v": va.astype(np.float16),
            }
            if with_vb:
                m["vb"] = va.astype(ml_dtypes.bfloat16)
            in_maps.append(m)
    return in_maps


def _assemble(results):
    out = np.empty((B, C, N), np.float32)
    for core in range(2 * B):
        b, h = divmod(core, 2)
        r = results[core]["out"]
        if r.shape == (CP1, MLOC):
            half = r[:C] / r[C:C + 1]
        else:  # flip_av: [128 m-rows, NMT * 4 * 65]
            rr = r.reshape(P, NMT, 4, CP1)
            vals = rr[:, :, :, :C] / rr[:, :, :, C:]     # [mrow, t, mb, o]
            half = np.transpose(vals, (3, 1, 2, 0)).reshape(C, MLOC)
        out[b, :, h * MLOC:(h + 1) * MLOC] = half
    return out


def run(x, Wq, Wk, Wv, trace=False, **trace_kwargs):
    nc = _get_nc()
    res = run_bass_kernel_spmd(
        nc,
        _make_in_maps(x, Wq, Wk, Wv, **_in_map_kwargs(BEST)),
        core_ids=list(range(2 * B)),
        trace=trace,
        **trace_kwargs,
    )
    return _assemble(res.results), res


def kernel(x, Wq, Wk, Wv):
    out, _ = run(x, Wq, Wk, Wv, trace=False)
    return out
